# revision 41
# baseline (speedup 1.0000x reference)
"""Trainium2 Bass kernel for DenseGINConv (batch of dense graphs).

Reference computation (per graph b):
    agg  = adj[b] @ x[b]                      # [N, F_IN]
    h    = (1 + eps) * x[b] + agg
    h    = relu(h @ W1 + b1) @ W2 + b2        # 2-layer MLP per node
    out  = where(mask[b, :, None], h, 0)

Sharding: pure data parallel — the batch dim B=64 is split 8 ways across the
8 NeuronCores (8 graphs per core); MLP weights and eps are replicated.

Per-core pipeline (v3 — 3-queue adj stream, bf16 aggregation):
  * adj[b] streams as 2MB half-graph DMAs cycled over all three DGE queues
    (ADJ_RINGS): 50% on the gpsimd SWDGE queue, which casts fp32 -> bf16
    *during* the DMA, and 25% each on the two HWDGE rings (sync/scalar) as
    fp32. One queue alone caps at ~260 GB/s; three together sustain
    ~295 GB/s/core of HBM traffic under 8-core contention (HBM reads stay
    fp32 — the 32MB/core roofline).
  * Each 128x128 tile is transposed on the TensorEngine (identity matmul,
    exact): bf16 chunks at 1 cycle/row, fp32 chunks at 2 (the fp32->bf16
    cast is absorbed by the PSUM evict copy; an explicit DVE pre-cast
    measured slower). The baseline's all-fp32 PE pipeline (~31k
    cycles/graph) was the real bottleneck at ~2.4GHz with HAM downclock;
    this one is ~21k cycles/graph and stays sub-critical.
  * PSUM evicts to bf16 adjT strips alternate the Vector/Scalar engines.
  * The (1+eps)*x self-loop is folded in by adding (1+eps)*I to the diagonal
    128x128 block of adjT on the Vector engine during evict — no extra
    TensorE work (the fp32r diagonal matmuls of v1 cost 4 cycles/row).
  * Aggregation in transposed feature space: aggT[f,i] accumulates
    x_bf16^T (stationary) @ adjT (streamed, N=512) into fp32 PSUM.
  * MLP stays fp32r: W1/W2 stationary, biases as per-partition activation
    biases, relu on the Scalar engine.
  * Output transposed back per 128-block on TensorE; node mask applied as a
    per-partition scalar multiply during PSUM evict; stores alternate the
    two HWDGE rings.

Measured (For_i repeat-difference, steady state): ~116 us/core vs the
155.6 us baseline; scale-relative max error vs the fp32 reference ~2.3e-3
(bf16 rounding of the aggregation operands; harness gate 2e-2).
"""

import numpy as np
from contextlib import ExitStack

B, N, F_IN, F_HID, F_OUT = 64, 1024, 64, 128, 64
N_CORES = 8
BPC = B // N_CORES  # graphs per core
P = 128
NT = N // P  # node tiles per graph
HALF = 512

_CACHE = {}

# Ring assignment for the 2MB half-graph adj chunks, cycled over chunks.
# gpsimd (SWDGE) casts fp32->bf16 during the DMA; sync/scalar (the two
# HWDGE rings) deliver fp32, transposed at 2 cycles/row instead of 1.
ADJ_RINGS = ("gpsimd", "sync", "gpsimd", "scalar")


def _build_nc(repeat=1, loop=0):
    """loop>0 wraps the body in a hardware For_i loop executing it `loop`
    times — used by the timing harness to scale device work without
    scaling the instruction count (neuronx-cc compile is the bottleneck
    on this box). The per-iteration back-edge drain cost is identical
    for any `repeat`, so differencing two NEFFs with the same `loop` and
    different `repeat` isolates the pipelined per-body time."""
    import concourse.mybir as mybir
    import concourse.tile as tile
    from concourse import bacc
    from concourse.masks import make_identity

    fp32 = mybir.dt.float32
    fp32r = mybir.dt.float32r
    bf16 = mybir.dt.bfloat16
    AF = mybir.ActivationFunctionType

    nc = bacc.Bacc(
        "TRN2", target_bir_lowering=False, debug=False, num_devices=N_CORES
    )
    x_d = nc.declare_dram_parameter("x", [BPC, N, F_IN], fp32, isOutput=False)
    adj_d = nc.declare_dram_parameter("adj", [BPC, N, N], fp32, isOutput=False)
    # mask is pre-packed on the host to the SBUF layout [p, b*NT+t] =
    # mask[b, t*128+p] so it loads as 128 contiguous 64B descriptors
    # instead of 8192 one-byte ones.
    mask_d = nc.declare_dram_parameter(
        "mask", [P, BPC * NT], mybir.dt.uint8, isOutput=False
    )
    w1_d = nc.declare_dram_parameter("W1", [F_IN, F_HID], fp32, isOutput=False)
    b1_d = nc.declare_dram_parameter("b1", [F_HID, 1], fp32, isOutput=False)
    w2_d = nc.declare_dram_parameter("W2", [F_HID, F_OUT], fp32, isOutput=False)
    b2_d = nc.declare_dram_parameter("b2", [F_OUT, 1], fp32, isOutput=False)
    eps_d = nc.declare_dram_parameter("eps", [1, 1], fp32, isOutput=False)
    out_d = nc.declare_dram_parameter("out", [BPC, N, F_OUT], fp32, isOutput=True)

    with tile.TileContext(nc) as tc:
        with ExitStack() as ctx:
            const = ctx.enter_context(tc.tile_pool(name="const", bufs=1))
            ident = const.tile([P, P], bf16)
            make_identity(nc, ident[:])

            # Const loads are deferred until after the first adj DMA is
            # queued so the small transfers don't delay the critical stream.
            w1_ld = const.tile([F_IN, F_HID], fp32)
            w1_sb = const.tile([F_IN, F_HID], fp32r)
            w2_ld = const.tile([F_HID, F_OUT], fp32)
            w2_sb = const.tile([F_HID, F_OUT], fp32r)
            b1_sb = const.tile([F_HID, 1], fp32)
            b2_sb = const.tile([F_OUT, 1], fp32)
            eps_sb = const.tile([1, 1], fp32)
            ones_sb = const.tile([1, P], fp32)

            # Working pools
            xp = ctx.enter_context(tc.tile_pool(name="xp", bufs=2))
            halfp = ctx.enter_context(tc.tile_pool(name="halfp", bufs=6))
            adjTp = ctx.enter_context(tc.tile_pool(name="adjTp", bufs=3))
            hp = ctx.enter_context(tc.tile_pool(name="hp", bufs=2))
            a1p = ctx.enter_context(tc.tile_pool(name="a1p", bufs=2))
            z2p = ctx.enter_context(tc.tile_pool(name="z2p", bufs=2))
            outp = ctx.enter_context(tc.tile_pool(name="outp", bufs=2))
            ps_a = ctx.enter_context(tc.tile_pool(name="ps_a", bufs=4, space="PSUM"))
            ps_o = ctx.enter_context(tc.tile_pool(name="ps_o", bufs=1, space="PSUM"))
            ps_b = ctx.enter_context(tc.tile_pool(name="ps_b", bufs=3, space="PSUM"))

            c_sb = const.tile([P, 1], fp32)
            ci_sb = const.tile([P, P], bf16)
            ci32_sb = const.tile([P, P], fp32)
            identf = const.tile([P, P], fp32r)
            ident32 = const.tile([P, P], fp32)

            mask_u8 = const.tile([P, BPC * NT], mybir.dt.uint8)
            mask_f = const.tile([P, BPC * NT], fp32)
            mask_loaded = [False]

            def load_mask():
                nc.sync.dma_start(out=w1_ld[:], in_=w1_d[:])
                nc.vector.tensor_copy(w1_sb[:], w1_ld[:])
                nc.sync.dma_start(out=w2_ld[:], in_=w2_d[:])
                nc.vector.tensor_copy(w2_sb[:], w2_ld[:])
                nc.sync.dma_start(out=b1_sb[:], in_=b1_d[:])
                nc.sync.dma_start(out=b2_sb[:], in_=b2_d[:])
                nc.sync.dma_start(out=eps_sb[:], in_=eps_d[:])
                nc.vector.memset(ones_sb[:], 1.0)
                # fp32r identity for the output transposes and fp32 identity
                # for the fp32 adj transposes (matmul requires fp32 operands
                # to pair) — exact copies of the bf16 ident.
                nc.vector.tensor_copy(identf[:], ident[:])
                nc.vector.tensor_copy(ident32[:], ident[:])
                # c = 1 + eps broadcast to 128 partitions via a K=1 matmul,
                # then cI = (1+eps) * I for the diagonal fold-in.
                c_ps = ps_a.tile([P, 1], fp32, tag="ps_tr")
                nc.tensor.matmul(
                    c_ps[:], ones_sb[:], eps_sb[:], start=True, stop=True
                )
                nc.scalar.add(c_sb[:], c_ps[:], 1.0)
                nc.vector.tensor_scalar_mul(ci_sb[:], ident[:], c_sb[:, 0:1])
                nc.vector.tensor_scalar_mul(ci32_sb[:], ident32[:], c_sb[:, 0:1])
                nc.scalar.dma_start(out=mask_u8[:], in_=mask_d[:])
                nc.vector.tensor_copy(mask_f[:], mask_u8[:])
                mask_loaded[0] = True

            def emit_body():
              for b in [g for _ in range(repeat) for g in range(BPC)]:
                # adjT strip for the whole graph, bf16 [j_part, jt, i]
                adjT = adjTp.tile([P, NT * N], bf16, tag="adjT")
                adjT3 = adjT[:].rearrange("p (j i) -> p j i", j=NT)

                x_ld = xp.tile([P, NT * F_IN], fp32, tag="x_ld")
                x_bf = xp.tile([P, NT * F_IN], bf16, tag="x_bf")

                for ih in range(2):
                    # 2MB of adj rows (i in [512*ih, 512*ih+512)). SWDGE
                    # chunks are cast to bf16 by the DMA datapath; HWDGE
                    # chunks arrive fp32 (HWDGE cannot cast).
                    ring = ADJ_RINGS[(b * 2 + ih) % len(ADJ_RINGS)]
                    hdt = bf16 if ring == "gpsimd" else fp32
                    half = halfp.tile([P, 4 * N], hdt, tag="half")
                    getattr(nc, ring).dma_start(
                        out=half[:].rearrange("p (r j) -> p r j", r=4),
                        in_=adj_d[b, ih * HALF : (ih + 1) * HALF, :].rearrange(
                            "(r p) j -> p r j", p=P
                        ),
                    )
                    if ih == 0:
                        # x staged via HWDGE behind the adj stream
                        nc.scalar.dma_start(
                            out=x_ld[:].rearrange("p (t f) -> p t f", t=NT),
                            in_=x_d[b].rearrange("(t p) f -> p t f", p=P),
                        )
                        nc.vector.tensor_copy(x_bf[:], x_ld[:])

                    if not mask_loaded[0]:
                        # consts must exist before the first diag fold-in,
                        # but their DMAs queue behind the first adj transfer
                        load_mask()

                    # ---- transpose this half's 32 tiles into adjT.
                    # bf16 (SWDGE) halves transpose at 1 cycle/row, fp32
                    # (HWDGE) halves at 2 — the PE absorbs the fp32->bf16
                    # cast during the PSUM evict copy. (An explicit DVE cast
                    # before the transpose measured slower: it serializes
                    # DMA->cast->transpose and contends with the evicts.)
                    h_ident = ident if hdt == bf16 else ident32
                    h_ci = ci_sb if hdt == bf16 else ci32_sb
                    for r in range(4):
                        it = 4 * ih + r
                        for jh in range(2):
                            ps_tr = ps_a.tile([P, 4 * P], hdt, tag="ps_tr")
                            for k in range(4):
                                jt = jh * 4 + k
                                nc.tensor.transpose(
                                    ps_tr[:, k * P : (k + 1) * P],
                                    half[:, r * N + jt * P : r * N + (jt + 1) * P],
                                    h_ident[:],
                                )
                            dest = adjT3[
                                :, jh * 4 : (jh + 1) * 4, it * P : (it + 1) * P
                            ]
                            src = ps_tr[:].rearrange("p (k i) -> p k i", k=4)
                            if (it + jh) % 2 == 0:
                                nc.vector.tensor_copy(dest, src)
                            else:
                                nc.scalar.copy(dest, src)
                            if jh == it // 4:
                                # diagonal block: adjT[it,it] += (1+eps)*I
                                k = it % 4
                                nc.vector.tensor_add(
                                    adjT3[:, it, it * P : (it + 1) * P],
                                    ps_tr[:, k * P : (k + 1) * P],
                                    h_ci[:],
                                )

                    # ---- aggregation for this i-half (contract over all jt)
                    lo = ih * HALF
                    agg = ps_b.tile([F_IN, HALF], fp32, tag="ps_big")
                    for jt in range(NT):
                        nc.tensor.matmul(
                            agg[:],
                            x_bf[:, jt * F_IN : (jt + 1) * F_IN],
                            adjT3[:, jt, lo : lo + HALF],
                            start=(jt == 0),
                            stop=(jt == NT - 1),
                        )

                    hT = hp.tile([F_IN, HALF], fp32r, tag="hT")
                    if ih == 0:
                        nc.vector.tensor_copy(hT[:], agg[:])
                    else:
                        nc.scalar.copy(hT[:], agg[:])

                    # ---- MLP (+relu/b1, then +b2)
                    z1 = ps_b.tile([F_HID, HALF], fp32, tag="ps_big")
                    nc.tensor.matmul(z1[:], w1_sb[:], hT[:], start=True, stop=True)
                    a1 = a1p.tile([F_HID, HALF], fp32r, tag="a1")
                    nc.scalar.activation(a1[:], z1[:], AF.Relu, bias=b1_sb[:, 0:1])
                    z2 = ps_b.tile([F_OUT, HALF], fp32, tag="ps_big")
                    nc.tensor.matmul(z2[:], w2_sb[:], a1[:], start=True, stop=True)
                    z2_sb = z2p.tile([F_OUT, HALF], fp32r, tag="z2_sb")
                    nc.scalar.activation(z2_sb[:], z2[:], AF.Identity, bias=b2_sb[:, 0:1])

                    # ---- transpose back + mask + store this half
                    out_sb = outp.tile([P, 4 * F_OUT], fp32, tag="out_sb")
                    ps_ot = ps_o.tile([P, 4 * F_OUT], fp32r, tag="ps_ot")
                    for k in range(4):
                        it = 4 * ih + k
                        nc.tensor.transpose(
                            ps_ot[:, k * F_OUT : (k + 1) * F_OUT],
                            z2_sb[:, k * P : (k + 1) * P],
                            identf[0:F_OUT, 0:F_OUT],
                        )
                        nc.vector.tensor_scalar_mul(
                            out_sb[:, k * F_OUT : (k + 1) * F_OUT],
                            ps_ot[:, k * F_OUT : (k + 1) * F_OUT],
                            mask_f[:, b * NT + it : b * NT + it + 1],
                        )
                    eng = nc.sync if ih == 0 else nc.scalar
                    eng.dma_start(
                        out=out_d[b, lo : lo + HALF, :].rearrange(
                            "(t p) f -> p t f", p=P
                        ),
                        in_=out_sb[:].rearrange("p (t f) -> p t f", t=4),
                    )

            if loop:
                with tc.For_i(0, loop, 1):
                    emit_body()
            else:
                emit_body()

    nc.compile()
    return nc


def _get_nc(repeat=1, loop=0):
    key = ("nc", repeat, loop)
    if key not in _CACHE:
        _CACHE[key] = _build_nc(repeat, loop)
    return _CACHE[key]


def _make_in_maps(inputs):
    x = np.asarray(inputs["x"], dtype=np.float32)
    adj = np.asarray(inputs["adj"], dtype=np.float32)
    # pack mask to the SBUF layout [p, b*NT + t] = mask[b, t*P + p]
    mask_u8 = (
        np.asarray(inputs["mask"])
        .astype(np.uint8)
        .reshape(B, NT, P)
        .transpose(2, 0, 1)
    )
    W1 = np.ascontiguousarray(np.asarray(inputs["W1"], dtype=np.float32))
    b1 = np.asarray(inputs["b1"], dtype=np.float32).reshape(F_HID, 1)
    W2 = np.ascontiguousarray(np.asarray(inputs["W2"], dtype=np.float32))
    b2 = np.asarray(inputs["b2"], dtype=np.float32).reshape(F_OUT, 1)
    eps = np.asarray(inputs["eps"], dtype=np.float32).reshape(1, 1)

    in_maps = []
    for c in range(N_CORES):
        sl = slice(c * BPC, (c + 1) * BPC)
        in_maps.append(
            {
                "x": np.ascontiguousarray(x[sl]),
                "adj": np.ascontiguousarray(adj[sl]),
                "mask": np.ascontiguousarray(mask_u8[:, sl, :].reshape(P, BPC * NT)),
                "W1": W1,
                "b1": b1,
                "W2": W2,
                "b2": b2,
                "eps": eps,
            }
        )
    return in_maps


def kernel(x, adj, mask, W1, b1, W2, b2, eps):
    from concourse.bass_utils import run_bass_kernel_spmd

    nc = _get_nc()
    in_maps = _make_in_maps(
        dict(x=x, adj=adj, mask=mask, W1=W1, b1=b1, W2=W2, b2=b2, eps=eps)
    )
    res = run_bass_kernel_spmd(nc, in_maps, list(range(N_CORES)))
    out = np.concatenate(
        [res.results[c]["out"] for c in range(N_CORES)], axis=0
    )
    return out


# revision 42
# speedup vs baseline: 1.7252x; 1.7252x over previous
"""Trainium2 Bass kernel for DenseGINConv (batch of dense graphs).

Reference computation (per graph b):
    agg  = adj[b] @ x[b]                      # [N, F_IN]
    h    = (1 + eps) * x[b] + agg
    h    = relu(h @ W1 + b1) @ W2 + b2        # 2-layer MLP per node
    out  = where(mask[b, :, None], h, 0)

Sharding: pure data parallel — the batch dim B=64 is split 8 ways across the
8 NeuronCores (8 graphs per core); MLP weights and eps are replicated.

Per-core pipeline (v3 — 3-queue adj stream, bf16 aggregation):
  * adj[b] streams as 2MB half-graph DMAs cycled over all three DGE queues
    (ADJ_RINGS): 50% on the gpsimd SWDGE queue, which casts fp32 -> bf16
    *during* the DMA, and 25% each on the two HWDGE rings (sync/scalar) as
    fp32. One queue alone caps at ~260 GB/s; three together sustain
    ~295 GB/s/core of HBM traffic under 8-core contention (HBM reads stay
    fp32 — the 32MB/core roofline).
  * Each 128x128 tile is transposed on the TensorEngine (identity matmul,
    exact): bf16 chunks at 1 cycle/row, fp32 chunks at 2 (the fp32->bf16
    cast is absorbed by the PSUM evict copy; an explicit DVE pre-cast
    measured slower). The baseline's all-fp32 PE pipeline (~31k
    cycles/graph) was the real bottleneck at ~2.4GHz with HAM downclock;
    this one is ~21k cycles/graph and stays sub-critical.
  * PSUM evicts to bf16 adjT strips alternate the Vector/Scalar engines.
  * The (1+eps)*x self-loop is folded in by adding (1+eps)*I to the diagonal
    128x128 block of adjT on the Vector engine during evict — no extra
    TensorE work (the fp32r diagonal matmuls of v1 cost 4 cycles/row).
  * Aggregation in transposed feature space: aggT[f,i] accumulates
    x_bf16^T (stationary) @ adjT (streamed, N=512) into fp32 PSUM.
  * MLP stays fp32r: W1/W2 stationary, biases as per-partition activation
    biases, relu on the Scalar engine.
  * Output transposed back per 128-block on TensorE; node mask applied as a
    per-partition scalar multiply during PSUM evict; stores alternate the
    two HWDGE rings.

Measured (For_i repeat-difference, steady state): ~116 us/core vs the
155.6 us baseline; scale-relative max error vs the fp32 reference ~2.3e-3
(bf16 rounding of the aggregation operands; harness gate 2e-2).
"""

import numpy as np
from contextlib import ExitStack

B, N, F_IN, F_HID, F_OUT = 64, 1024, 64, 128, 64
N_CORES = 8
BPC = B // N_CORES  # graphs per core
P = 128
NT = N // P  # node tiles per graph
HALF = 512

_CACHE = {}

# Ring assignment for the 2MB half-graph adj chunks, cycled over chunks.
# gpsimd (SWDGE) casts fp32->bf16 during the DMA; sync/scalar (the two
# HWDGE rings) deliver fp32, transposed at 2 cycles/row instead of 1.
ADJ_RINGS = ("gpsimd", "sync", "gpsimd", "scalar")


def _build_nc(repeat=1, loop=0):
    """loop>0 wraps the body in a hardware For_i loop executing it `loop`
    times — used by the timing harness to scale device work without
    scaling the instruction count (neuronx-cc compile is the bottleneck
    on this box). The per-iteration back-edge drain cost is identical
    for any `repeat`, so differencing two NEFFs with the same `loop` and
    different `repeat` isolates the pipelined per-body time."""
    import concourse.mybir as mybir
    import concourse.tile as tile
    from concourse import bacc
    from concourse.masks import make_identity

    fp32 = mybir.dt.float32
    fp32r = mybir.dt.float32r
    bf16 = mybir.dt.bfloat16
    AF = mybir.ActivationFunctionType

    nc = bacc.Bacc(
        "TRN2", target_bir_lowering=False, debug=False, num_devices=N_CORES
    )
    x_d = nc.declare_dram_parameter("x", [BPC, N, F_IN], fp32, isOutput=False)
    adj_d = nc.declare_dram_parameter("adj", [BPC, N, N], fp32, isOutput=False)
    # mask is pre-packed on the host to the SBUF layout [p, b*NT+t] =
    # mask[b, t*128+p] so it loads as 128 contiguous 64B descriptors
    # instead of 8192 one-byte ones.
    mask_d = nc.declare_dram_parameter(
        "mask", [P, BPC * NT], mybir.dt.uint8, isOutput=False
    )
    w1_d = nc.declare_dram_parameter("W1", [F_IN, F_HID], fp32, isOutput=False)
    b1_d = nc.declare_dram_parameter("b1", [F_HID, 1], fp32, isOutput=False)
    w2_d = nc.declare_dram_parameter("W2", [F_HID, F_OUT], fp32, isOutput=False)
    b2_d = nc.declare_dram_parameter("b2", [F_OUT, 1], fp32, isOutput=False)
    eps_d = nc.declare_dram_parameter("eps", [1, 1], fp32, isOutput=False)
    out_d = nc.declare_dram_parameter("out", [BPC, N, F_OUT], fp32, isOutput=True)

    with tile.TileContext(nc) as tc:
        with ExitStack() as ctx:
            const = ctx.enter_context(tc.tile_pool(name="const", bufs=1))
            ident = const.tile([P, P], bf16)
            make_identity(nc, ident[:])

            # Const loads are deferred until after the first adj DMA is
            # queued so the small transfers don't delay the critical stream.
            w1_ld = const.tile([F_IN, F_HID], fp32)
            w1_sb = const.tile([F_IN, F_HID], fp32r)
            w2_ld = const.tile([F_HID, F_OUT], fp32)
            w2_sb = const.tile([F_HID, F_OUT], fp32r)
            b1_sb = const.tile([F_HID, 1], fp32)
            b2_sb = const.tile([F_OUT, 1], fp32)
            eps_sb = const.tile([1, 1], fp32)
            ones_sb = const.tile([1, P], fp32)

            # Working pools
            xp = ctx.enter_context(tc.tile_pool(name="xp", bufs=2))
            halfp = ctx.enter_context(tc.tile_pool(name="halfp", bufs=6))
            # bufs=2 is load-bearing: bufs=3 measured 1.7x SLOWER (200us vs
            # 116us) — the extra 16KB/partition of SBUF evidently upsets the
            # SWDGE scratch / port balance. Do not "deepen" this pool.
            adjTp = ctx.enter_context(tc.tile_pool(name="adjTp", bufs=2))
            hp = ctx.enter_context(tc.tile_pool(name="hp", bufs=2))
            a1p = ctx.enter_context(tc.tile_pool(name="a1p", bufs=2))
            z2p = ctx.enter_context(tc.tile_pool(name="z2p", bufs=2))
            outp = ctx.enter_context(tc.tile_pool(name="outp", bufs=2))
            ps_a = ctx.enter_context(tc.tile_pool(name="ps_a", bufs=4, space="PSUM"))
            ps_o = ctx.enter_context(tc.tile_pool(name="ps_o", bufs=1, space="PSUM"))
            ps_b = ctx.enter_context(tc.tile_pool(name="ps_b", bufs=3, space="PSUM"))

            c_sb = const.tile([P, 1], fp32)
            ci_sb = const.tile([P, P], bf16)
            ci32_sb = const.tile([P, P], fp32)
            identf = const.tile([P, P], fp32r)
            ident32 = const.tile([P, P], fp32)

            mask_u8 = const.tile([P, BPC * NT], mybir.dt.uint8)
            mask_f = const.tile([P, BPC * NT], fp32)
            mask_loaded = [False]

            def load_mask():
                nc.sync.dma_start(out=w1_ld[:], in_=w1_d[:])
                nc.vector.tensor_copy(w1_sb[:], w1_ld[:])
                nc.sync.dma_start(out=w2_ld[:], in_=w2_d[:])
                nc.vector.tensor_copy(w2_sb[:], w2_ld[:])
                nc.sync.dma_start(out=b1_sb[:], in_=b1_d[:])
                nc.sync.dma_start(out=b2_sb[:], in_=b2_d[:])
                nc.sync.dma_start(out=eps_sb[:], in_=eps_d[:])
                nc.vector.memset(ones_sb[:], 1.0)
                # fp32r identity for the output transposes and fp32 identity
                # for the fp32 adj transposes (matmul requires fp32 operands
                # to pair) — exact copies of the bf16 ident.
                nc.vector.tensor_copy(identf[:], ident[:])
                nc.vector.tensor_copy(ident32[:], ident[:])
                # c = 1 + eps broadcast to 128 partitions via a K=1 matmul,
                # then cI = (1+eps) * I for the diagonal fold-in.
                c_ps = ps_a.tile([P, 1], fp32, tag="ps_tr")
                nc.tensor.matmul(
                    c_ps[:], ones_sb[:], eps_sb[:], start=True, stop=True
                )
                nc.scalar.add(c_sb[:], c_ps[:], 1.0)
                nc.vector.tensor_scalar_mul(ci_sb[:], ident[:], c_sb[:, 0:1])
                nc.vector.tensor_scalar_mul(ci32_sb[:], ident32[:], c_sb[:, 0:1])
                nc.scalar.dma_start(out=mask_u8[:], in_=mask_d[:])
                nc.vector.tensor_copy(mask_f[:], mask_u8[:])
                mask_loaded[0] = True

            def emit_body():
              for b in [g for _ in range(repeat) for g in range(BPC)]:
                # adjT strip for the whole graph, bf16 [j_part, jt, i]
                adjT = adjTp.tile([P, NT * N], bf16, tag="adjT")
                adjT3 = adjT[:].rearrange("p (j i) -> p j i", j=NT)

                x_ld = xp.tile([P, NT * F_IN], fp32, tag="x_ld")
                x_bf = xp.tile([P, NT * F_IN], bf16, tag="x_bf")

                for ih in range(2):
                    # 2MB of adj rows (i in [512*ih, 512*ih+512)). SWDGE
                    # chunks are cast to bf16 by the DMA datapath; HWDGE
                    # chunks arrive fp32 (HWDGE cannot cast).
                    ring = ADJ_RINGS[(b * 2 + ih) % len(ADJ_RINGS)]
                    hdt = bf16 if ring == "gpsimd" else fp32
                    half = halfp.tile([P, 4 * N], hdt, tag="half")
                    getattr(nc, ring).dma_start(
                        out=half[:].rearrange("p (r j) -> p r j", r=4),
                        in_=adj_d[b, ih * HALF : (ih + 1) * HALF, :].rearrange(
                            "(r p) j -> p r j", p=P
                        ),
                    )
                    if ih == 0:
                        # x staged via HWDGE behind the adj stream
                        nc.scalar.dma_start(
                            out=x_ld[:].rearrange("p (t f) -> p t f", t=NT),
                            in_=x_d[b].rearrange("(t p) f -> p t f", p=P),
                        )
                        nc.vector.tensor_copy(x_bf[:], x_ld[:])

                    if not mask_loaded[0]:
                        # consts must exist before the first diag fold-in,
                        # but their DMAs queue behind the first adj transfer
                        load_mask()

                    # ---- transpose this half's 32 tiles into adjT.
                    # bf16 (SWDGE) halves transpose at 1 cycle/row, fp32
                    # (HWDGE) halves at 2 — the PE absorbs the fp32->bf16
                    # cast during the PSUM evict copy. (An explicit DVE cast
                    # before the transpose measured slower: it serializes
                    # DMA->cast->transpose and contends with the evicts.)
                    h_ident = ident if hdt == bf16 else ident32
                    h_ci = ci_sb if hdt == bf16 else ci32_sb
                    for r in range(4):
                        it = 4 * ih + r
                        for jh in range(2):
                            ps_tr = ps_a.tile([P, 4 * P], hdt, tag="ps_tr")
                            for k in range(4):
                                jt = jh * 4 + k
                                nc.tensor.transpose(
                                    ps_tr[:, k * P : (k + 1) * P],
                                    half[:, r * N + jt * P : r * N + (jt + 1) * P],
                                    h_ident[:],
                                )
                            dest = adjT3[
                                :, jh * 4 : (jh + 1) * 4, it * P : (it + 1) * P
                            ]
                            src = ps_tr[:].rearrange("p (k i) -> p k i", k=4)
                            if (it + jh) % 2 == 0:
                                nc.vector.tensor_copy(dest, src)
                            else:
                                nc.scalar.copy(dest, src)
                            if jh == it // 4:
                                # diagonal block: adjT[it,it] += (1+eps)*I
                                k = it % 4
                                nc.vector.tensor_add(
                                    adjT3[:, it, it * P : (it + 1) * P],
                                    ps_tr[:, k * P : (k + 1) * P],
                                    h_ci[:],
                                )

                    # ---- aggregation for this i-half (contract over all jt)
                    lo = ih * HALF
                    agg = ps_b.tile([F_IN, HALF], fp32, tag="ps_big")
                    for jt in range(NT):
                        nc.tensor.matmul(
                            agg[:],
                            x_bf[:, jt * F_IN : (jt + 1) * F_IN],
                            adjT3[:, jt, lo : lo + HALF],
                            start=(jt == 0),
                            stop=(jt == NT - 1),
                        )

                    hT = hp.tile([F_IN, HALF], fp32r, tag="hT")
                    if ih == 0:
                        nc.vector.tensor_copy(hT[:], agg[:])
                    else:
                        nc.scalar.copy(hT[:], agg[:])

                    # ---- MLP (+relu/b1, then +b2)
                    z1 = ps_b.tile([F_HID, HALF], fp32, tag="ps_big")
                    nc.tensor.matmul(z1[:], w1_sb[:], hT[:], start=True, stop=True)
                    a1 = a1p.tile([F_HID, HALF], fp32r, tag="a1")
                    nc.scalar.activation(a1[:], z1[:], AF.Relu, bias=b1_sb[:, 0:1])
                    z2 = ps_b.tile([F_OUT, HALF], fp32, tag="ps_big")
                    nc.tensor.matmul(z2[:], w2_sb[:], a1[:], start=True, stop=True)
                    z2_sb = z2p.tile([F_OUT, HALF], fp32r, tag="z2_sb")
                    nc.scalar.activation(z2_sb[:], z2[:], AF.Identity, bias=b2_sb[:, 0:1])

                    # ---- transpose back + mask + store this half
                    out_sb = outp.tile([P, 4 * F_OUT], fp32, tag="out_sb")
                    ps_ot = ps_o.tile([P, 4 * F_OUT], fp32r, tag="ps_ot")
                    for k in range(4):
                        it = 4 * ih + k
                        nc.tensor.transpose(
                            ps_ot[:, k * F_OUT : (k + 1) * F_OUT],
                            z2_sb[:, k * P : (k + 1) * P],
                            identf[0:F_OUT, 0:F_OUT],
                        )
                        nc.vector.tensor_scalar_mul(
                            out_sb[:, k * F_OUT : (k + 1) * F_OUT],
                            ps_ot[:, k * F_OUT : (k + 1) * F_OUT],
                            mask_f[:, b * NT + it : b * NT + it + 1],
                        )
                    eng = nc.sync if ih == 0 else nc.scalar
                    eng.dma_start(
                        out=out_d[b, lo : lo + HALF, :].rearrange(
                            "(t p) f -> p t f", p=P
                        ),
                        in_=out_sb[:].rearrange("p (t f) -> p t f", t=4),
                    )

            if loop:
                with tc.For_i(0, loop, 1):
                    emit_body()
            else:
                emit_body()

    nc.compile()
    return nc


def _get_nc(repeat=1, loop=0):
    key = ("nc", repeat, loop)
    if key not in _CACHE:
        _CACHE[key] = _build_nc(repeat, loop)
    return _CACHE[key]


def _make_in_maps(inputs):
    x = np.asarray(inputs["x"], dtype=np.float32)
    adj = np.asarray(inputs["adj"], dtype=np.float32)
    # pack mask to the SBUF layout [p, b*NT + t] = mask[b, t*P + p]
    mask_u8 = (
        np.asarray(inputs["mask"])
        .astype(np.uint8)
        .reshape(B, NT, P)
        .transpose(2, 0, 1)
    )
    W1 = np.ascontiguousarray(np.asarray(inputs["W1"], dtype=np.float32))
    b1 = np.asarray(inputs["b1"], dtype=np.float32).reshape(F_HID, 1)
    W2 = np.ascontiguousarray(np.asarray(inputs["W2"], dtype=np.float32))
    b2 = np.asarray(inputs["b2"], dtype=np.float32).reshape(F_OUT, 1)
    eps = np.asarray(inputs["eps"], dtype=np.float32).reshape(1, 1)

    in_maps = []
    for c in range(N_CORES):
        sl = slice(c * BPC, (c + 1) * BPC)
        in_maps.append(
            {
                "x": np.ascontiguousarray(x[sl]),
                "adj": np.ascontiguousarray(adj[sl]),
                "mask": np.ascontiguousarray(mask_u8[:, sl, :].reshape(P, BPC * NT)),
                "W1": W1,
                "b1": b1,
                "W2": W2,
                "b2": b2,
                "eps": eps,
            }
        )
    return in_maps


def kernel(x, adj, mask, W1, b1, W2, b2, eps):
    from concourse.bass_utils import run_bass_kernel_spmd

    nc = _get_nc()
    in_maps = _make_in_maps(
        dict(x=x, adj=adj, mask=mask, W1=W1, b1=b1, W2=W2, b2=b2, eps=eps)
    )
    res = run_bass_kernel_spmd(nc, in_maps, list(range(N_CORES)))
    out = np.concatenate(
        [res.results[c]["out"] for c in range(N_CORES)], axis=0
    )
    return out
